# revision 12
# baseline (speedup 1.0000x reference)
"""Trainium2 Bass kernel for nn_CustomRNN_88871463289370.

Reference computation (B=1024, T=256, H=512, HORIZON=24):
    h_0 = 0
    h_{t+1} = tanh(outer(x[:, t], Wx_w) + h_t @ Wh_w.T + (Wx_b + Wh_b))
    out = h_T @ fc_w.T + fc_b                      # [B, 24]

Strategy (data-parallel over batch, 8 cores x 128 rows each):
  * Feature-major ("transposed") on-chip layout: hT[k] tiles are
    [128 hidden-features (partition), 128 batch (free)], k = 0..3.
  * Per step, per output chunk m (4 chunks of 128 hidden units):
      psum[m] = WxB_chunk.T @ [x_t ; ones]        (K=2 matmul: x-outer + bias)
              + sum_k WhT[k, m-chunk].T @ hT[k]   (4 K=128 matmuls, PSUM accum)
    then hT'[m] = tanh(psum[m]) on ScalarE.
  * Two merged [128, 256] Tanh activations per step (chunks m01 / m23) so
    the ACT engine work (~1.0us/step) hides under PE work (~1.5us/step).
  * Matmuls are emitted "k-phase-major" (all k in {0,1} for every m, then
    all k in {2,3}) so the first half of the next step only depends on the
    first ACT of this step -- shortens the serial chain.
  * x rows are DMA'd one step at a time straight from DRAM into tiny
    [2, 128] tiles ([x_t ; ones]); these only depend on kernel inputs so
    Tile's scheduler hoists them far ahead of the compute.
  * Final projection: 4 K=128 matmuls into a [24, 128] PSUM tile plus a
    per-partition-bias Identity activation.

All host-side reshaping/transposition/casting happens in kernel() below;
the device kernel sees pre-massaged tensors.
"""

import numpy as np
import ml_dtypes

HIDDEN = 512
HORIZON = 24
B_FULL = 1024
T_FULL = 256
N_CORES = 8
B_CORE = B_FULL // N_CORES  # 128
KC = HIDDEN // 128          # 4 chunks of the hidden dim

_COMPILED = {}


def build_kernel(T=T_FULL, use_bf16=True):
    """Build the Bass module. Returns (nc, input_names)."""
    import concourse.bass as bass
    import concourse.mybir as mybir
    import concourse.tile as tile
    from concourse.bass import ts

    dt = mybir.dt.bfloat16 if use_bf16 else mybir.dt.float32
    f32 = mybir.dt.float32

    nc = bass.Bass("TRN2", target_bir_lowering=False, debug=False,
                   num_devices=N_CORES)

    # ---- DRAM I/O (per-core shapes; host pre-massages layouts) ----
    # x2T[0, t] = x[:, t], x2T[1, t] = ones; shape [2, T, B_CORE]
    x2T_d = nc.dram_tensor("x2T", [2, T, B_CORE], dt, kind="ExternalInput").ap()
    # WxB[0] = Wx_w, WxB[1] = Wx_b + Wh_b, shape [2, H]
    wxb_d = nc.dram_tensor("wxb", [2, HIDDEN], dt, kind="ExternalInput").ap()
    # WhT arranged [128, KC, H]: whT[p, k, m] = Wh_w[m, k*128+p]
    whT_d = nc.dram_tensor("whT", [128, KC, HIDDEN], dt, kind="ExternalInput").ap()
    # fcT arranged [128, KC, HORIZON]: fcT[p, k, n] = fc_w[n, k*128+p]
    fcT_d = nc.dram_tensor("fcT", [128, KC, HORIZON], dt, kind="ExternalInput").ap()
    # fc_b as column [HORIZON, 1] fp32
    fcb_d = nc.dram_tensor("fcb", [HORIZON, 1], f32, kind="ExternalInput").ap()
    # output [HORIZON, B_CORE] fp32 (host transposes/concats)
    out_d = nc.dram_tensor("out", [HORIZON, B_CORE], f32, kind="ExternalOutput").ap()

    with tile.TileContext(nc) as tc:
        with (
            tc.tile_pool(name="consts", bufs=1) as cpool,
            tc.tile_pool(name="h", bufs=2) as hpool,
            tc.tile_pool(name="ps", bufs=2, space="PSUM") as pspool,
            tc.tile_pool(name="fin", bufs=1) as finpool,
        ):
            # ---- load constants into SBUF ----
            # all x rows resident: [2, T, B] on partitions 0-1
            x2_sb = cpool.tile([2, T, B_CORE], dt)
            nc.sync.dma_start(x2_sb[:], x2T_d[:])
            wxb_sb = cpool.tile([2, HIDDEN], dt)
            nc.sync.dma_start(wxb_sb[:], wxb_d[:])
            whT_sb = cpool.tile([128, KC, HIDDEN], dt)
            nc.sync.dma_start(whT_sb[:], whT_d[:])
            fcT_sb = cpool.tile([128, KC, HORIZON], dt)
            nc.sync.dma_start(fcT_sb[:], fcT_d[:])
            fcb_sb = cpool.tile([HORIZON, 1], f32)
            nc.sync.dma_start(fcb_sb[:], fcb_d[:])
            # Touch fcb on ScalarE right away so the DMA wait lands here,
            # not on the final bias activation (which already carries a PE
            # wait; the AC instruction struct fits only one sync wait).
            fcb_scratch = cpool.tile([1, 1], f32)
            nc.scalar.activation(fcb_scratch[:], fcb_sb[0:1, 0:1],
                                 mybir.ActivationFunctionType.Identity)

            h01 = None  # [128, 256] tiles: hT chunks 0|1 and 2|3
            h23 = None

            for t in range(T):
                xr = x2_sb[:, t, :]

                psA = pspool.tile([128, 256], f32, tag="psA")
                psB = pspool.tile([128, 256], f32, tag="psB")
                ps_of = lambda m: (psA, psB)[m // 2][:, ts(m % 2, 128)]

                # start/stop are per PSUM *bank*: exactly one start=True on
                # the first matmul into each tile and one stop=True on the
                # last (start marks the whole 2KB bank pending-zero).
                # phase 0: x-outer + bias (K=2)
                for m in range(4):
                    nc.tensor.matmul(ps_of(m), wxb_sb[0:2, ts(m, 128)], xr,
                                     start=(m % 2 == 0),
                                     stop=(t == 0 and m % 2 == 1))
                if t > 0:
                    # phase 1: k in {0, 1} -> depends on h01 of prev step
                    for m in range(4):
                        for k in (0, 1):
                            nc.tensor.matmul(ps_of(m),
                                             whT_sb[:, k, ts(m, 128)],
                                             h01[:, ts(k, 128)],
                                             start=False, stop=False)
                    # phase 2: k in {2, 3} -> depends on h23 of prev step
                    for m in range(4):
                        for k in (2, 3):
                            nc.tensor.matmul(ps_of(m),
                                             whT_sb[:, k, ts(m, 128)],
                                             h23[:, ts(k - 2, 128)],
                                             start=False,
                                             stop=(k == 3 and m % 2 == 1))
                        if m == 1:
                            h01_new = hpool.tile([128, 256], dt, tag="h01")
                            nc.scalar.activation(
                                h01_new[:], psA[:],
                                mybir.ActivationFunctionType.Tanh)
                else:
                    h01_new = hpool.tile([128, 256], dt, tag="h01")
                    nc.scalar.activation(h01_new[:], psA[:],
                                         mybir.ActivationFunctionType.Tanh)
                h23_new = hpool.tile([128, 256], dt, tag="h23")
                nc.scalar.activation(h23_new[:], psB[:],
                                     mybir.ActivationFunctionType.Tanh)
                h01, h23 = h01_new, h23_new

            # ---- final projection: out[n, b] = sum_k fcT[k].T @ hT[k] + b ----
            ps_fc = pspool.tile([HORIZON, B_CORE], f32, tag="psA")
            hs = (h01, h01, h23, h23)
            for k in range(KC):
                nc.tensor.matmul(ps_fc[:], fcT_sb[:, k, :],
                                 hs[k][:, ts(k % 2, 128)],
                                 start=(k == 0), stop=(k == KC - 1))
            out_sb = finpool.tile([HORIZON, B_CORE], f32)
            nc.scalar.activation(out_sb[:], ps_fc[:],
                                 mybir.ActivationFunctionType.Identity,
                                 bias=fcb_sb[:])
            nc.sync.dma_start(out_d[:], out_sb[:])

    _strip_redundant_self_waits(nc)
    return nc


_SELF_SEM_PREFIX = {
    "InstActivation": "Activation",
    "InstMatmult": "PE",
    "InstLdweights": "PE",
    "InstTensorTensor": "DVE",
    "InstTensorScalarPtr": "DVE",
    "InstTensorCopy": "DVE",
}


def _strip_redundant_self_waits(nc):
    """Drop same-engine semaphore waits from instructions that carry more
    than one sync wait.

    Rationale: the HW engine instruction structs (MM/AC) hold only ONE
    sync-wait command; walrus refuses to codegen instructions with two.
    Tile emits a wait on the instruction's own engine sem for WAW/WAR on
    recycled tile-pool slots, but each engine executes its queue strictly
    in order, so ordering vs. its own earlier instructions is guaranteed
    without the wait.  Cross-engine waits are preserved; sem update counts
    are untouched (no other wait thresholds shift).
    """
    # Semaphore updated by the final DMA store of the "out" tensor; the
    # kernel-tail drain only genuinely needs this one (everything else is
    # transitively ordered: input DMAs -> compute -> final ACT -> out DMA).
    out_dma_sems = set()
    for b in nc.m.functions[0].blocks:
        for i in b.instructions:
            if type(i).__name__ != "InstDMACopy":
                continue
            names = [getattr(ap, "memref", "") for ap in i.outs]
            if "out" in names:
                si = i.sync_info
                if si:
                    out_dma_sems.update(u.ant_name for u in si.on_update)

    for b in nc.m.functions[0].blocks:
        for i in b.instructions:
            si = i.sync_info
            if si is None:
                continue
            ow = si.on_wait
            if len(ow) < 2:
                continue
            tname = type(i).__name__
            if tname == "InstDrain" and any(
                w.ant_name in out_dma_sems for w in ow
            ):
                si.on_wait = [w for w in ow if w.ant_name in out_dma_sems][:1]
                continue
            self_prefix = _SELF_SEM_PREFIX.get(tname)
            if self_prefix is None:
                continue
            kept = [w for w in ow if not w.ant_name.startswith(self_prefix)]
            if kept and len(kept) < len(ow):
                si.on_wait = kept


def _prep_inputs(x, Wx_w, Wx_b, Wh_w, Wh_b, fc_w, fc_b, T, use_bf16):
    """Host-side shard + layout massaging. Returns per-core input maps."""
    dt = ml_dtypes.bfloat16 if use_bf16 else np.float32
    bias = (Wx_b + Wh_b).astype(np.float32)

    wxb = np.stack([Wx_w.astype(np.float32), bias]).astype(dt)          # [2, H]
    whT = (Wh_w.T.astype(np.float32)
           .reshape(KC, 128, HIDDEN).transpose(1, 0, 2).copy().astype(dt))
    fcT = (fc_w.T.astype(np.float32)
           .reshape(KC, 128, HORIZON).transpose(1, 0, 2).copy().astype(dt))
    fcb = fc_b.astype(np.float32).reshape(HORIZON, 1).copy()

    in_maps = []
    for c in range(N_CORES):
        xs = x[c * B_CORE:(c + 1) * B_CORE, :T]                          # [128, T]
        x2T = np.empty((2, T, B_CORE), dtype=np.float32)
        x2T[0] = xs.T
        x2T[1] = 1.0
        in_maps.append({
            "x2T": x2T.astype(dt),
            "wxb": wxb,
            "whT": whT,
            "fcT": fcT,
            "fcb": fcb,
        })
    return in_maps


def kernel(x, Wx_w, Wx_b, Wh_w, Wh_b, fc_w, fc_b, _T=T_FULL, _bf16=True,
           _trace=False):
    from concourse.bass_utils import run_bass_kernel_spmd

    key = (_T, _bf16)
    if key not in _COMPILED:
        _COMPILED[key] = build_kernel(T=_T, use_bf16=_bf16)
    nc = _COMPILED[key]

    in_maps = _prep_inputs(x, Wx_w, Wx_b, Wh_w, Wh_b, fc_w, fc_b, _T, _bf16)
    res = run_bass_kernel_spmd(nc, in_maps, list(range(N_CORES)), trace=_trace)
    outs = [res.results[c]["out"] for c in range(N_CORES)]               # [24, 128] each
    full = np.concatenate(outs, axis=1).T.astype(np.float32).copy()      # [1024, 24]
    kernel._last_result = res
    return full
